# revision 10
# baseline (speedup 1.0000x reference)
"""Trainium2 Bass kernel for nn_FFFFanout (moe_routing tree-MLP).

Contract: kernel(**inputs) takes FULL unsharded numpy inputs
  oldx  [2, 2048, 1024] f32
  W_in  [21840, 1024]   f32
  b_in  [21840]         f32
  W_out [1024, 21840]   f32
returns [2, 2048, 1024] f32.

Strategy: data-parallel over the 4096 flattened tokens -> 512 per core on 8
NeuronCores. Device layout is "f-major": dev_hid(f, p, g) = f*5632 + p*1408 + g
with G padded 1365 -> 1408 so each (p, g)-plane tile aligns to 128 partitions.
This makes the group-of-4 argmax an elementwise max across 4 f-plane tiles,
the tree mask multiply partition-aligned, and both matmuls contraction-friendly
(activations live as [hid, tokens]).

Per core:
  phase A: logits.T tiles [128h, 512tok] = W_inT.T @ x.T (PSUM, K=1024 in 8
           chunks), gelu+bias on ACT, spill act tiles to DRAM. Routing region
           (g < 341, i.e. tree levels 0..4) in fp32, rest in fp32r.
           Group argmax index dec in {0..3} from DVE compares (first-max
           tie-break identical to np.argmax).
  phase B: gather-free tree cascade: child(g, m) = 4g+1+m, so
           sel[d+1][4j+f] = sel[d][j] * (dec[j] == f), levels written into a
           [5632, tok] mask via partition-interleave SBUF DMAs.
  phase C: masked = act * mask (DVE, out fp32r), out.T [1024d, 512tok]
           accumulated over all 176 h-tiles in all 8 PSUM banks, fp32r matmuls.
"""
import sys

if "/opt/trn_rl_repo" not in sys.path:
    sys.path.insert(0, "/opt/trn_rl_repo")

from contextlib import ExitStack

import numpy as np

import concourse.bass as bass  # noqa: F401
import concourse.mybir as mybir
import concourse.tile as tile
from concourse import bacc
from concourse.bass_utils import run_bass_kernel_spmd

F32 = mybir.dt.float32
F32R = mybir.dt.float32r
BF16 = mybir.dt.bfloat16

D = 1024
P = 4
DEPTH = 5
FAN = 4
G = 1365
HID = 21840
Gp = 1408            # 11 * 128
Sp = P * Gp          # 5632  (one f-plane)
HIDp = FAN * Sp      # 22528
NT = HIDp // 128     # 176 h-tiles
NPG = Sp // 128      # 44 (p,g) tiles per f-plane
T = 512              # tokens per core
NCORES = 8
KC = D // 128        # 8 contraction chunks

ROUTE_SUBS = 3                  # g < 341 lives in subtiles 0..2 of each p
ROUTE_G = 341                   # groups 0..340 are tree levels 0..4

# routing h-tiles ordered (p, sub, f): the four f-planes of one (p,g)-tile are
# adjacent because the group argmax consumes all four together
ROUTE_TILES = [f * NPG + p * 11 + sub
               for p in range(P) for sub in range(ROUTE_SUBS) for f in range(FAN)]
ROUTE_SET = set(ROUTE_TILES)
NONROUTE_TILES = [t for t in range(NT) if t not in ROUTE_SET]
# phase A / phase C production+consumption order
TILE_ORDER = ROUTE_TILES + NONROUTE_TILES


def _platform(d):
    return (FAN ** d - 1) // 3


def _segments(q0, q1, *bases):
    """Split [q0, q1) at every multiple of 128 relative to each base offset."""
    cuts = {q0, q1}
    for b in bases:
        k = b + ((q0 - b) // 128 + 1) * 128
        while k < q1:
            cuts.add(k)
            k += 128
    cs = sorted(cuts)
    return list(zip(cs[:-1], cs[1:]))


def build_nc():
    nc = bacc.Bacc("TRN2", target_bir_lowering=False, debug=False,
                   num_devices=NCORES)

    xT32 = nc.dram_tensor("xT32", [D, T], F32, kind="ExternalInput").ap()
    xTr = nc.dram_tensor("xTr", [D, T], F32R, kind="ExternalInput").ap()
    Wroute = nc.dram_tensor("Wroute", [D, len(ROUTE_TILES) * 128], F32,
                            kind="ExternalInput").ap()
    Wfull = nc.dram_tensor("Wfull", [D, HIDp], F32R, kind="ExternalInput").ap()
    bvec = nc.dram_tensor("bvec", [HIDp], F32, kind="ExternalInput").ap()
    WoT = nc.dram_tensor("WoT", [HIDp, D], F32R, kind="ExternalInput").ap()
    outT = nc.dram_tensor("outT", [D, T], F32, kind="ExternalOutput").ap()

    with tile.TileContext(nc) as tc, ExitStack() as top:
        perm = top.enter_context(tc.tile_pool(name="perm", bufs=1))
        dramp = top.enter_context(tc.tile_pool(name="dram", bufs=1, space="DRAM"))

        mask = perm.tile([128, NPG, T], BF16)        # 5.8 MB
        dec = perm.tile([128, P * ROUTE_SUBS, T], F32)   # 3.1 MB
        bt = perm.tile([128, NT], F32)
        nc.sync.dma_start(bt[:], bvec.rearrange("(t p) -> p t", p=128))
        nc.vector.memset(mask[:], 0.0)

        act_scr = [dramp.tile([128, T], F32, tag=f"as{t}", name=f"as{t}")
                   for t in range(NT)]

        # ---------------- phase A: matmul1 + gelu + dec ----------------
        with ExitStack() as pa:
            constp = pa.enter_context(tc.tile_pool(name="xconst", bufs=1))
            wrp = pa.enter_context(tc.tile_pool(name="wroute", bufs=2))
            wfp = pa.enter_context(tc.tile_pool(name="wfull", bufs=3))
            actp = pa.enter_context(tc.tile_pool(name="act", bufs=2))
            tmpp = pa.enter_context(tc.tile_pool(name="tmp", bufs=2))
            psA = pa.enter_context(tc.tile_pool(name="psA", bufs=2, space="PSUM"))

            xt32 = constp.tile([128, KC, T], F32)
            nc.sync.dma_start(xt32[:], xT32.rearrange("(c p) t -> p c t", p=128))
            xtr = constp.tile([128, KC, T], F32R)
            nc.sync.dma_start(xtr[:], xTr.rearrange("(c p) t -> p c t", p=128))

            route_acts = {}
            for rt, t in enumerate(ROUTE_TILES):
                w = wrp.tile([128, KC, 128], F32, tag="wr")
                nc.sync.dma_start(
                    w[:],
                    Wroute[:, rt * 128:(rt + 1) * 128]
                    .rearrange("(c p) h -> p c h", p=128))
                ps = psA.tile([128, T], F32, tag="ps")
                for c in range(KC):
                    nc.tensor.matmul(ps[:], w[:, c, :], xt32[:, c, :],
                                     start=(c == 0), stop=(c == KC - 1))
                f = t // NPG
                a = actp.tile([128, T], F32, tag=f"act{f}")
                nc.scalar.activation(a[:], ps[:],
                                     mybir.ActivationFunctionType.Gelu,
                                     bias=bt[:, t:t + 1], scale=1.0)
                nc.sync.dma_start(act_scr[t][:], a[:])
                route_acts[t] = a

                rem = t % NPG
                p, sub = divmod(rem, 11)
                if f == FAN - 1:
                    # all four f-planes of (p, sub) live -> dec
                    a0 = route_acts.pop(0 * NPG + rem)
                    a1 = route_acts.pop(1 * NPG + rem)
                    a2 = route_acts.pop(2 * NPG + rem)
                    a3 = route_acts.pop(3 * NPG + rem)
                    m01 = tmpp.tile([128, T], F32, tag="m01")
                    m23 = tmpp.tile([128, T], F32, tag="m23")
                    nc.vector.tensor_tensor(m01[:], a0[:], a1[:], mybir.AluOpType.max)
                    nc.vector.tensor_tensor(m23[:], a2[:], a3[:], mybir.AluOpType.max)
                    b1t = tmpp.tile([128, T], mybir.dt.uint8, tag="b1")
                    nc.vector.tensor_tensor(b1t[:], m23[:], m01[:], mybir.AluOpType.is_gt)
                    b01 = tmpp.tile([128, T], F32, tag="b01")
                    nc.vector.tensor_tensor(b01[:], a1[:], a0[:], mybir.AluOpType.is_gt)
                    b23 = tmpp.tile([128, T], F32, tag="b23")
                    nc.vector.tensor_tensor(b23[:], a3[:], a2[:], mybir.AluOpType.is_gt)
                    b0t = tmpp.tile([128, T], F32, tag="b0")
                    nc.vector.select(b0t[:], b1t[:], b23[:], b01[:])
                    nc.vector.scalar_tensor_tensor(
                        dec[:, p * ROUTE_SUBS + sub, :], b1t[:], 2.0, b0t[:],
                        op0=mybir.AluOpType.mult, op1=mybir.AluOpType.add)

            for t in NONROUTE_TILES:
                w = wfp.tile([128, KC, 128], F32R, tag="wf")
                nc.sync.dma_start(
                    w[:],
                    Wfull[:, t * 128:(t + 1) * 128]
                    .rearrange("(c p) h -> p c h", p=128))
                ps = psA.tile([128, T], F32, tag="ps")
                for c in range(KC):
                    nc.tensor.matmul(ps[:], w[:, c, :], xtr[:, c, :],
                                     start=(c == 0), stop=(c == KC - 1))
                a = actp.tile([128, T], F32, tag="actn")
                nc.scalar.activation(a[:], ps[:],
                                     mybir.ActivationFunctionType.Gelu,
                                     bias=bt[:, t:t + 1], scale=1.0)
                nc.sync.dma_start(act_scr[t][:], a[:])

            # ---------------- phase B: cascade ----------------
            # Engine APs need 32-aligned partition starts: product ops run on
            # 32-aligned padded windows (junk lanes never scattered out).
            # prodI rows are q-aligned (same partition as dec/mask source).
            cascp = pa.enter_context(tc.tile_pool(name="casc", bufs=1))
            prodI = cascp.tile([128, ROUTE_SUBS, FAN, T], BF16)

            for p in range(P):
                base = p * Gp
                # level 0: mask[q=base]=1; rows 1..31 get junk 1.0s that every
                # level-d scatter fully overwrites before level d+1 reads them
                nc.vector.memset(mask[0:32, p * 11, :], 1.0)
                for d in range(DEPTH):
                    plat, platn, n = _platform(d), _platform(d + 1), FAN ** d
                    q0 = base + plat
                    # products (dec[q] == f) * sel[q] -> prodI[q, f]
                    for (qa, qb) in _segments(q0, q0 + n, 0):
                        sub = (qa - base) // 128
                        pr_a, pr_b = qa % 128, (qb - 1) % 128 + 1
                        al_a = pr_a - pr_a % 32
                        al_b = min(128, -(-pr_b // 32) * 32)
                        for f in range(FAN):
                            nc.vector.scalar_tensor_tensor(
                                prodI[al_a:al_b, sub, f, :],
                                dec[al_a:al_b, p * ROUTE_SUBS + sub, :],
                                float(f),
                                mask[al_a:al_b, p * 11 + sub, :],
                                op0=mybir.AluOpType.is_equal,
                                op1=mybir.AluOpType.mult)
                    # scatter prodI -> mask at q' = base + platn + 4j + f
                    c0 = base + platn
                    for r in range(c0 // 128, (c0 + 4 * n - 1) // 128 + 1):
                        lo, hi = max(c0, 128 * r), min(c0 + 4 * n, 128 * (r + 1))
                        for f in range(FAN):
                            ja = max(0, -(-(lo - c0 - f) // 4))
                            jb = min(n, (hi - 1 - c0 - f) // 4 + 1)
                            if ja >= jb:
                                continue
                            for (j1, j2) in _segments(ja, jb, -q0):
                                qsrc = q0 + j1
                                sub = (qsrc - base) // 128
                                pd = (c0 + 4 * j1 + f) % 128
                                nc.sync.dma_start(
                                    mask[pd:pd + 4 * (j2 - j1 - 1) + 1:4, r, :],
                                    prodI[qsrc % 128:qsrc % 128 + (j2 - j1),
                                          sub, f, :])

        # ---------------- phase C: mask-mult + matmul2 ----------------
        with ExitStack() as pc:
            wop = pc.enter_context(tc.tile_pool(name="wo", bufs=3))
            actc = pc.enter_context(tc.tile_pool(name="actc", bufs=4))
            mskp = pc.enter_context(tc.tile_pool(name="msk", bufs=4))
            outp = pc.enter_context(tc.tile_pool(name="outp", bufs=1))
            psC = pc.enter_context(tc.tile_pool(name="psC", bufs=1, space="PSUM"))

            cps = psC.tile([128, KC, T], F32)
            for i, t in enumerate(TILE_ORDER):
                a = actc.tile([128, T], F32, tag="a")
                nc.sync.dma_start(a[:], act_scr[t][:])
                m = mskp.tile([128, T], F32R, tag="m")
                nc.vector.tensor_tensor(m[:], a[:], mask[:, t % NPG, :],
                                        mybir.AluOpType.mult)
                wo = wop.tile([128, D], F32R, tag="wo")
                nc.sync.dma_start(wo[:], WoT[t * 128:(t + 1) * 128, :])
                for dd in range(KC):
                    nc.tensor.matmul(cps[:, dd, :],
                                     wo[:, dd * 128:(dd + 1) * 128], m[:],
                                     start=(i == 0), stop=(i == NT - 1))

            osb = outp.tile([128, KC, T], F32)
            for dd in range(KC):
                nc.vector.tensor_copy(osb[:, dd, :], cps[:, dd, :])
            nc.sync.dma_start(outT.rearrange("(c p) t -> p c t", p=128), osb[:])

    nc.compile()
    return nc


_NC_CACHE = None


def _get_nc():
    global _NC_CACHE
    if _NC_CACHE is None:
        _NC_CACHE = build_nc()
    return _NC_CACHE


def _prep_inputs(oldx, W_in, b_in, W_out):
    x = np.ascontiguousarray(np.asarray(oldx, np.float32).reshape(-1, D))
    xT = np.ascontiguousarray(x.T)                      # [D, B]

    Wr = np.asarray(W_in, np.float32).reshape(P, G, FAN, D)
    W_dev = np.zeros((FAN, P, Gp, D), np.float32)
    W_dev[:, :, :G, :] = Wr.transpose(2, 0, 1, 3)
    W_dev = W_dev.reshape(HIDp, D)
    WT_dev = np.ascontiguousarray(W_dev.T)              # [D, HIDp]

    # routing columns, ordered like ROUTE_TILES
    cols = []
    for t in ROUTE_TILES:
        cols.append(WT_dev[:, t * 128:(t + 1) * 128])
    Wroute = np.ascontiguousarray(np.concatenate(cols, axis=1))

    br = np.asarray(b_in, np.float32).reshape(P, G, FAN)
    b_dev = np.zeros((FAN, P, Gp), np.float32)
    b_dev[:, :, :G] = br.transpose(2, 0, 1)
    b_dev = np.ascontiguousarray(b_dev.reshape(HIDp))

    Wo = np.asarray(W_out, np.float32).reshape(D, P, G, FAN)
    Wo_dev = np.zeros((FAN, P, Gp, D), np.float32)
    Wo_dev[:, :, :G, :] = Wo.transpose(3, 1, 2, 0)
    WoT_dev = np.ascontiguousarray(Wo_dev.reshape(HIDp, D))

    return xT, Wroute, WT_dev, b_dev, WoT_dev


def run(oldx, W_in, b_in, W_out, trace=False):
    nc = _get_nc()
    xT, Wroute, WT_dev, b_dev, WoT_dev = _prep_inputs(oldx, W_in, b_in, W_out)

    in_maps = []
    for c in range(NCORES):
        xs = np.ascontiguousarray(xT[:, c * T:(c + 1) * T])
        in_maps.append({
            "xT32": xs, "xTr": xs,
            "Wroute": Wroute, "Wfull": WT_dev,
            "bvec": b_dev, "WoT": WoT_dev,
        })
    res = run_bass_kernel_spmd(nc, in_maps, list(range(NCORES)), trace=trace)

    outT = np.concatenate([res.results[c]["outT"] for c in range(NCORES)],
                          axis=1)                        # [D, B]
    out = np.ascontiguousarray(outT.T).reshape(np.asarray(oldx).shape)
    return out.astype(np.float32), res


def kernel(oldx, W_in, b_in, W_out):
    out, _ = run(oldx, W_in, b_in, W_out, trace=False)
    return out


# revision 12
# speedup vs baseline: 1.1057x; 1.1057x over previous
"""Trainium2 Bass kernel for nn_FFFFanout (moe_routing tree-MLP).

Contract: kernel(**inputs) takes FULL unsharded numpy inputs
  oldx  [2, 2048, 1024] f32
  W_in  [21840, 1024]   f32
  b_in  [21840]         f32
  W_out [1024, 21840]   f32
returns [2, 2048, 1024] f32.

Strategy: data-parallel over the 4096 flattened tokens -> 512 per core on 8
NeuronCores. Device layout is "f-major": dev_hid(f, p, g) = f*5632 + p*1408 + g
with G padded 1365 -> 1408 so each (p, g)-plane tile aligns to 128 partitions.
This makes the group-of-4 argmax an elementwise max across 4 f-plane tiles,
the tree mask multiply partition-aligned, and both matmuls contraction-friendly
(activations live as [hid, tokens]).

Per core:
  phase A: logits.T tiles [128h, 512tok] = W_inT.T @ x.T (PSUM, K=1024 in 8
           chunks), gelu+bias on ACT, spill act tiles to DRAM. Routing region
           (g < 341, i.e. tree levels 0..4) in fp32, rest in fp32r.
           Group argmax index dec in {0..3} from DVE compares (first-max
           tie-break identical to np.argmax).
  phase B: gather-free tree cascade: child(g, m) = 4g+1+m, so
           sel[d+1][4j+f] = sel[d][j] * (dec[j] == f), levels written into a
           [5632, tok] mask via partition-interleave SBUF DMAs.
  phase C: masked = act * mask (DVE, out fp32r), out.T [1024d, 512tok]
           accumulated over all 176 h-tiles in all 8 PSUM banks, fp32r matmuls.
"""
import sys

if "/opt/trn_rl_repo" not in sys.path:
    sys.path.insert(0, "/opt/trn_rl_repo")

from contextlib import ExitStack

import numpy as np

import concourse.bass as bass  # noqa: F401
import concourse.mybir as mybir
import concourse.tile as tile
from concourse import bacc
from concourse.bass_utils import run_bass_kernel_spmd

F32 = mybir.dt.float32
F32R = mybir.dt.float32r
BF16 = mybir.dt.bfloat16

D = 1024
P = 4
DEPTH = 5
FAN = 4
G = 1365
HID = 21840
Gp = 1408            # 11 * 128
Sp = P * Gp          # 5632  (one f-plane)
HIDp = FAN * Sp      # 22528
NT = HIDp // 128     # 176 h-tiles
NPG = Sp // 128      # 44 (p,g) tiles per f-plane
T = 512              # tokens per core
NCORES = 8
KC = D // 128        # 8 contraction chunks

ROUTE_SUBS = 3                  # g < 341 lives in subtiles 0..2 of each p
ROUTE_G = 341                   # groups 0..340 are tree levels 0..4

# routing h-tiles ordered (p, sub, f): the four f-planes of one (p,g)-tile are
# adjacent because the group argmax consumes all four together
ROUTE_TILES = [f * NPG + p * 11 + sub
               for p in range(P) for sub in range(ROUTE_SUBS) for f in range(FAN)]
ROUTE_SET = set(ROUTE_TILES)
NONROUTE_TILES = [t for t in range(NT) if t not in ROUTE_SET]
# phase A / phase C production+consumption order
TILE_ORDER = ROUTE_TILES + NONROUTE_TILES


def _platform(d):
    return (FAN ** d - 1) // 3


def _segments(q0, q1, *bases):
    """Split [q0, q1) at every multiple of 128 relative to each base offset."""
    cuts = {q0, q1}
    for b in bases:
        k = b + ((q0 - b) // 128 + 1) * 128
        while k < q1:
            cuts.add(k)
            k += 128
    cs = sorted(cuts)
    return list(zip(cs[:-1], cs[1:]))


def build_nc():
    nc = bacc.Bacc("TRN2", target_bir_lowering=False, debug=False,
                   num_devices=NCORES)

    xT32 = nc.dram_tensor("xT32", [D, T], F32, kind="ExternalInput").ap()
    xTr = nc.dram_tensor("xTr", [D, T], F32R, kind="ExternalInput").ap()
    Wroute = nc.dram_tensor("Wroute", [D, len(ROUTE_TILES) * 128], F32,
                            kind="ExternalInput").ap()
    Wfull = nc.dram_tensor("Wfull", [D, HIDp], F32R, kind="ExternalInput").ap()
    bvec = nc.dram_tensor("bvec", [128, NT], F32, kind="ExternalInput").ap()
    WoT = nc.dram_tensor("WoT", [HIDp, D], F32R, kind="ExternalInput").ap()
    outT = nc.dram_tensor("outT", [D, T], F32, kind="ExternalOutput").ap()

    with tile.TileContext(nc) as tc, ExitStack() as top:
        perm = top.enter_context(tc.tile_pool(name="perm", bufs=1))
        dramp = top.enter_context(tc.tile_pool(name="dram", bufs=1, space="DRAM"))

        mask = perm.tile([128, NPG, T], BF16)        # 5.8 MB
        dec = perm.tile([128, P * ROUTE_SUBS, T], F32)   # 3.1 MB
        bt = perm.tile([128, NT], F32)
        nc.gpsimd.dma_start(bt[:], bvec[:])
        nc.gpsimd.memset(mask[:], 0.0)

        act_scr = [dramp.tile([128, T], F32, tag=f"as{t}", name=f"as{t}")
                   for t in range(NT)]

        # ---------------- phase A: matmul1 + gelu + dec ----------------
        with ExitStack() as pa:
            constp = pa.enter_context(tc.tile_pool(name="xconst", bufs=1))
            wrp = pa.enter_context(tc.tile_pool(name="wroute", bufs=3))
            wfp = pa.enter_context(tc.tile_pool(name="wfull", bufs=5))
            actp = pa.enter_context(tc.tile_pool(name="act", bufs=2))
            tmpp = pa.enter_context(tc.tile_pool(name="tmp", bufs=2))
            psA = pa.enter_context(tc.tile_pool(name="psA", bufs=3, space="PSUM"))

            xt32 = constp.tile([128, KC, T], F32)
            nc.gpsimd.dma_start(xt32[:], xT32.rearrange("(c p) t -> p c t", p=128))
            xtr = constp.tile([128, KC, T], F32R)
            nc.gpsimd.dma_start(xtr[:], xTr.rearrange("(c p) t -> p c t", p=128))

            route_acts = {}
            for rt, t in enumerate(ROUTE_TILES):
                w = wrp.tile([128, KC, 128], F32, tag="wr")
                nc.sync.dma_start(
                    w[:],
                    Wroute[:, rt * 128:(rt + 1) * 128]
                    .rearrange("(c p) h -> p c h", p=128))
                ps = psA.tile([128, T], F32, tag="ps")
                for c in range(KC):
                    nc.tensor.matmul(ps[:], w[:, c, :], xt32[:, c, :],
                                     start=(c == 0), stop=(c == KC - 1))
                f = t // NPG
                a = actp.tile([128, T], F32, tag=f"act{f}")
                nc.scalar.activation(a[:], ps[:],
                                     mybir.ActivationFunctionType.Gelu,
                                     bias=bt[:, t:t + 1], scale=1.0)
                nc.sync.dma_start(act_scr[t][:], a[:])
                route_acts[t] = a

                rem = t % NPG
                p, sub = divmod(rem, 11)
                if f == FAN - 1:
                    # all four f-planes of (p, sub) live -> dec
                    a0 = route_acts.pop(0 * NPG + rem)
                    a1 = route_acts.pop(1 * NPG + rem)
                    a2 = route_acts.pop(2 * NPG + rem)
                    a3 = route_acts.pop(3 * NPG + rem)
                    m01 = tmpp.tile([128, T], F32, tag="m01")
                    m23 = tmpp.tile([128, T], F32, tag="m23")
                    nc.vector.tensor_tensor(m01[:], a0[:], a1[:], mybir.AluOpType.max)
                    nc.vector.tensor_tensor(m23[:], a2[:], a3[:], mybir.AluOpType.max)
                    b1t = tmpp.tile([128, T], mybir.dt.uint8, tag="b1")
                    nc.vector.tensor_tensor(b1t[:], m23[:], m01[:], mybir.AluOpType.is_gt)
                    b01 = tmpp.tile([128, T], F32, tag="b01")
                    nc.vector.tensor_tensor(b01[:], a1[:], a0[:], mybir.AluOpType.is_gt)
                    b23 = tmpp.tile([128, T], F32, tag="b23")
                    nc.vector.tensor_tensor(b23[:], a3[:], a2[:], mybir.AluOpType.is_gt)
                    b0t = tmpp.tile([128, T], F32, tag="b0")
                    nc.vector.select(b0t[:], b1t[:], b23[:], b01[:])
                    nc.vector.scalar_tensor_tensor(
                        dec[:, p * ROUTE_SUBS + sub, :], b1t[:], 2.0, b0t[:],
                        op0=mybir.AluOpType.mult, op1=mybir.AluOpType.add)

            for t in NONROUTE_TILES:
                w = wfp.tile([128, KC, 128], F32R, tag="wf")
                nc.sync.dma_start(
                    w[:],
                    Wfull[:, t * 128:(t + 1) * 128]
                    .rearrange("(c p) h -> p c h", p=128))
                ps = psA.tile([128, T], F32, tag="ps")
                for c in range(KC):
                    nc.tensor.matmul(ps[:], w[:, c, :], xtr[:, c, :],
                                     start=(c == 0), stop=(c == KC - 1))
                a = actp.tile([128, T], F32, tag="actn")
                nc.scalar.activation(a[:], ps[:],
                                     mybir.ActivationFunctionType.Gelu,
                                     bias=bt[:, t:t + 1], scale=1.0)
                nc.sync.dma_start(act_scr[t][:], a[:])

            # ---------------- phase B: cascade ----------------
            # Engine APs need 32-aligned partition starts: product ops run on
            # 32-aligned padded windows (junk lanes never scattered out).
            # prodI rows are q-aligned (same partition as dec/mask source).
            cascp = pa.enter_context(tc.tile_pool(name="casc", bufs=1))
            prodI = cascp.tile([128, ROUTE_SUBS, FAN, T], BF16)

            for p in range(P):
                base = p * Gp
                # level 0: mask[q=base]=1; rows 1..31 get junk 1.0s that every
                # level-d scatter fully overwrites before level d+1 reads them
                nc.vector.memset(mask[0:32, p * 11, :], 1.0)
                for d in range(DEPTH):
                    plat, platn, n = _platform(d), _platform(d + 1), FAN ** d
                    q0 = base + plat
                    # products (dec[q] == f) * sel[q] -> prodI[q, f]
                    for (qa, qb) in _segments(q0, q0 + n, 0):
                        sub = (qa - base) // 128
                        pr_a, pr_b = qa % 128, (qb - 1) % 128 + 1
                        al_a = pr_a - pr_a % 32
                        al_b = min(128, -(-pr_b // 32) * 32)
                        for f in range(FAN):
                            nc.vector.scalar_tensor_tensor(
                                prodI[al_a:al_b, sub, f, :],
                                dec[al_a:al_b, p * ROUTE_SUBS + sub, :],
                                float(f),
                                mask[al_a:al_b, p * 11 + sub, :],
                                op0=mybir.AluOpType.is_equal,
                                op1=mybir.AluOpType.mult)
                    # scatter prodI -> mask at q' = base + platn + 4j + f
                    c0 = base + platn
                    for r in range(c0 // 128, (c0 + 4 * n - 1) // 128 + 1):
                        lo, hi = max(c0, 128 * r), min(c0 + 4 * n, 128 * (r + 1))
                        for f in range(FAN):
                            ja = max(0, -(-(lo - c0 - f) // 4))
                            jb = min(n, (hi - 1 - c0 - f) // 4 + 1)
                            if ja >= jb:
                                continue
                            for (j1, j2) in _segments(ja, jb, -q0):
                                qsrc = q0 + j1
                                sub = (qsrc - base) // 128
                                pd = (c0 + 4 * j1 + f) % 128
                                nc.sync.dma_start(
                                    mask[pd:pd + 4 * (j2 - j1 - 1) + 1:4, r, :],
                                    prodI[qsrc % 128:qsrc % 128 + (j2 - j1),
                                          sub, f, :])

        # ---------------- phase C: mask-mult + matmul2 ----------------
        with ExitStack() as pc:
            wop = pc.enter_context(tc.tile_pool(name="wo", bufs=4))
            actc = pc.enter_context(tc.tile_pool(name="actc", bufs=6))
            mskp = pc.enter_context(tc.tile_pool(name="msk", bufs=6))
            outp = pc.enter_context(tc.tile_pool(name="outp", bufs=1))
            psC = pc.enter_context(tc.tile_pool(name="psC", bufs=1, space="PSUM"))

            cps = psC.tile([128, KC, T], F32)
            for i, t in enumerate(TILE_ORDER):
                a = actc.tile([128, T], F32, tag="a")
                nc.sync.dma_start(a[:], act_scr[t][:])
                m = mskp.tile([128, T], F32R, tag="m")
                nc.vector.tensor_tensor(m[:], a[:], mask[:, t % NPG, :],
                                        mybir.AluOpType.mult)
                wo = wop.tile([128, D], F32R, tag="wo")
                nc.sync.dma_start(wo[:], WoT[t * 128:(t + 1) * 128, :])
                for dd in range(KC):
                    nc.tensor.matmul(cps[:, dd, :],
                                     wo[:, dd * 128:(dd + 1) * 128], m[:],
                                     start=(i == 0), stop=(i == NT - 1))

            osb = outp.tile([128, KC, T], F32)
            for dd in range(KC):
                nc.vector.tensor_copy(osb[:, dd, :], cps[:, dd, :])
            nc.sync.dma_start(outT.rearrange("(c p) t -> p c t", p=128), osb[:])

    nc.compile()
    return nc


_NC_CACHE = None


def _get_nc():
    global _NC_CACHE
    if _NC_CACHE is None:
        _NC_CACHE = build_nc()
    return _NC_CACHE


def _prep_inputs(oldx, W_in, b_in, W_out):
    x = np.ascontiguousarray(np.asarray(oldx, np.float32).reshape(-1, D))
    xT = np.ascontiguousarray(x.T)                      # [D, B]

    Wr = np.asarray(W_in, np.float32).reshape(P, G, FAN, D)
    W_dev = np.zeros((FAN, P, Gp, D), np.float32)
    W_dev[:, :, :G, :] = Wr.transpose(2, 0, 1, 3)
    W_dev = W_dev.reshape(HIDp, D)
    WT_dev = np.ascontiguousarray(W_dev.T)              # [D, HIDp]

    # routing columns, ordered like ROUTE_TILES
    cols = []
    for t in ROUTE_TILES:
        cols.append(WT_dev[:, t * 128:(t + 1) * 128])
    Wroute = np.ascontiguousarray(np.concatenate(cols, axis=1))

    br = np.asarray(b_in, np.float32).reshape(P, G, FAN)
    b_dev = np.zeros((FAN, P, Gp), np.float32)
    b_dev[:, :, :G] = br.transpose(2, 0, 1)
    b_dev = np.ascontiguousarray(b_dev.reshape(HIDp).reshape(NT, 128).T)

    Wo = np.asarray(W_out, np.float32).reshape(D, P, G, FAN)
    Wo_dev = np.zeros((FAN, P, Gp, D), np.float32)
    Wo_dev[:, :, :G, :] = Wo.transpose(3, 1, 2, 0)
    WoT_dev = np.ascontiguousarray(Wo_dev.reshape(HIDp, D))

    return xT, Wroute, WT_dev, b_dev, WoT_dev


def run(oldx, W_in, b_in, W_out, trace=False):
    nc = _get_nc()
    xT, Wroute, WT_dev, b_dev, WoT_dev = _prep_inputs(oldx, W_in, b_in, W_out)

    in_maps = []
    for c in range(NCORES):
        xs = np.ascontiguousarray(xT[:, c * T:(c + 1) * T])
        in_maps.append({
            "xT32": xs, "xTr": xs,
            "Wroute": Wroute, "Wfull": WT_dev,
            "bvec": b_dev, "WoT": WoT_dev,
        })
    res = run_bass_kernel_spmd(nc, in_maps, list(range(NCORES)), trace=trace)

    outT = np.concatenate([res.results[c]["outT"] for c in range(NCORES)],
                          axis=1)                        # [D, B]
    out = np.ascontiguousarray(outT.T).reshape(np.asarray(oldx).shape)
    return out.astype(np.float32), res


def kernel(oldx, W_in, b_in, W_out):
    out, _ = run(oldx, W_in, b_in, W_out, trace=False)
    return out
